# revision 14
# baseline (speedup 1.0000x reference)
"""Bass/Tile kernel for masked dot-product attention on 8 Trainium2 cores.

Problem: queries/keys/values [128, 1024, 64] fp32, valid_lens [128] int32.
  out[b] = softmax(mask(Q K^T / 8, valid_lens[b])) @ V

Strategy:
  * Shard the 128 batch*heads across 8 cores, 16 head-slots per core.
    Heads are sorted by valid_len (descending) and dealt round-robin so
    every core gets the same per-slot chunk count -> one SPMD program.
  * Per head, only ceil(valid_len/128) key chunks contribute (the rest are
    fully masked -> softmax weight exactly 0), so the program is
    specialized to skip them (~45% of the work for uniform valid_lens).
  * Layout: compute S^T = K Q^T chunkwise on the PE ([128 k x 1024 q]),
    so the PV matmul can consume P^T directly as the moving operand.
    Masking + 1/sqrt(d) scaling + exp run as a single ScalarE activation
    (bias = per-partition mask column of 0 / -1e6; no max subtraction is
    needed: scores are bounded and exp(-1e6) underflows to exactly 0,
    matching the fp32 reference).
  * Softmax denominators come free: a ones-column is appended to V, so
    the PV accumulation produces [O^T ; sum_k P^T] in one pass.
    Normalization happens after a final PE transpose, where the
    denominator is a per-partition scalar.
  * Heads with valid_len == 0 (reference: uniform attention) are fixed up
    on the host with the exact reference semantics (mean of V).
"""

import math
from contextlib import ExitStack

import numpy as np

import concourse.bass as bass  # noqa: F401  (engine namespaces live on the nc)
import concourse.mybir as mybir
import concourse.tile as tile
from concourse import bacc
from concourse.bass_utils import run_bass_kernel_spmd
from concourse.masks import make_identity

BH, L, D = 128, 1024, 64
NCORES = 8
SLOTS = BH // NCORES  # 16
CHUNK = 128
NCH = L // CHUNK  # 8
MASK_VALUE = -1000000.0
F32 = mybir.dt.float32
MM_DT = mybir.dt.float16  # 1 cyc/row on PE, ~2^-11 operand quantization

_program_cache: dict = {}


def _build_program(m_list):
    nc = bacc.Bacc("TRN2", target_bir_lowering=False, debug=False)
    NP = SLOTS // 2
    q_d = nc.dram_tensor("q", [NP, L, 2 * D], F32, kind="ExternalInput").ap()
    k_d = nc.dram_tensor("k", [NP, L, 2 * D], F32, kind="ExternalInput").ap()
    v_d = nc.dram_tensor("v", [SLOTS, L, D], F32, kind="ExternalInput").ap()
    qscr = [
        nc.dram_tensor(f"qscr{p}", [L, 2 * D], MM_DT).ap() for p in range(NP)
    ]
    kscr = [
        nc.dram_tensor(f"kscr{p}", [L, 2 * D], MM_DT).ap() for p in range(NP)
    ]
    mb_d = nc.dram_tensor("mb", [CHUNK, SLOTS * NCH], F32, kind="ExternalInput").ap()
    o_d = nc.dram_tensor("o", [SLOTS, L, D], F32, kind="ExternalOutput").ap()

    Exp = mybir.ActivationFunctionType.Exp

    with tile.TileContext(nc) as tc, ExitStack() as ctx:
        const = ctx.enter_context(tc.tile_pool(name="const", bufs=1))
        ident = const.tile([128, 128], F32)
        make_identity(nc, ident)
        mb = const.tile([CHUNK, SLOTS * NCH], F32)
        nc.sync.dma_start(mb[:], mb_d[:])
        ones = const.tile([128, 1], F32)
        nc.gpsimd.memset(ones[:], 1.0)

        qpf_p = ctx.enter_context(tc.tile_pool(name="qpf", bufs=2))
        qpb_p = ctx.enter_context(tc.tile_pool(name="qpb", bufs=2))
        qt_p = ctx.enter_context(tc.tile_pool(name="qt", bufs=2))
        kt_p = ctx.enter_context(tc.tile_pool(name="kt", bufs=2))
        vnat_p = ctx.enter_context(tc.tile_pool(name="vnat", bufs=3))
        vp_p = ctx.enter_context(tc.tile_pool(name="vp", bufs=3))
        pt_p = ctx.enter_context(tc.tile_pool(name="pt", bufs=3))
        ot_p = ctx.enter_context(tc.tile_pool(name="ot", bufs=2))
        osb_p = ctx.enter_context(tc.tile_pool(name="osb", bufs=4))
        rec_p = ctx.enter_context(tc.tile_pool(name="rec", bufs=4))

        # PSUM: 8 banks. "s": S^T tiles + epilogue transposes (2 x 2 banks);
        # "ops": PV accumulators (2 x 2 banks).
        s_ps = ctx.enter_context(tc.tile_pool(name="s", bufs=2, space="PSUM"))
        o_ps = ctx.enter_context(tc.tile_pool(name="ops", bufs=2, space="PSUM"))

        # Dense matmul burst to flip the PE HAM clock-gate to full rate
        # (~3.4us of contiguous activity required) before real work starts.
        warm = const.tile([128, 512], MM_DT, tag="warm")
        nc.gpsimd.memset(warm[:], 0.5)
        wps = o_ps.tile([128, 512], F32, tag="ops")  # noqa
        for i in range(24):
            nc.tensor.matmul(
                wps[:], warm[:, 0:128], warm[:], start=True, stop=True
            )

        def transposed_load(src_pair, scr, nrows, tag):
            """DRAM f32 [nrows, 128] -> SBUF fp16 [128, nrows] transposed.

            Strided load -> DVE fp16 cast -> DRAM fp16 scratch -> XBAR DMA
            transpose. Row 0-63 of the result = head A's d dims, 64-127 = B's.
            """
            nch = nrows // CHUNK
            pf = qpf_p.tile([128, L * 2], F32, tag="pf")
            nc.sync.dma_start(
                pf[:, 0 : nch * 2 * D].rearrange("p (c d) -> p c d", d=2 * D),
                src_pair[0:nrows].rearrange("(c p) d -> p c d", p=CHUNK),
            )
            pb = qpb_p.tile([128, L * 2], MM_DT, tag="pb")
            nc.vector.tensor_copy(pb[:, 0 : nch * 2 * D], pf[:, 0 : nch * 2 * D])
            nc.sync.dma_start(
                scr[0:nrows].rearrange("(c p) d -> p c d", p=CHUNK),
                pb[:, 0 : nch * 2 * D].rearrange("p (c d) -> p c d", d=2 * D),
            )
            t = qt_p.tile([128, L], MM_DT, tag=tag)
            nc.sync.dma_start_transpose(t[:, 0:nrows], scr[0:nrows])
            return t

        for p in range(SLOTS // 2):
            ja, jb = 2 * p, 2 * p + 1
            ma, mb_n = m_list[ja], m_list[jb]
            mmax = max(ma, mb_n)

            qt = transposed_load(q_d[p], qscr[p], L, "qt")
            kt = transposed_load(k_d[p], kscr[p], mmax * CHUNK, "kt")

            # V (fp16 cast) with appended ones-column, per head.
            vps = []
            for j, m in ((ja, ma), (jb, mb_n)):
                vnat = vnat_p.tile([128, NCH * D], F32, tag="vnat")
                nc.sync.dma_start(
                    vnat[:, 0 : m * D].rearrange("p (c d) -> p c d", d=D),
                    v_d[j, 0 : m * CHUNK].rearrange("(c p) d -> p c d", p=CHUNK),
                )
                vp = vp_p.tile([128, NCH * (D + 1)], MM_DT, tag="vp")
                nc.vector.tensor_copy(
                    vp[:].rearrange("p (c e) -> p c e", e=D + 1)[:, 0:m, 0:D],
                    vnat[:, 0 : m * D].rearrange("p (c d) -> p c d", d=D),
                )
                for c in range(m):
                    base = c * (D + 1)
                    nc.vector.tensor_copy(vp[:, base + D : base + D + 1], ones[:])
                vps.append(vp)

            # Main loop: row-tiled paired QK^T, exp, PV accumulation.
            opsum_a = o_ps.tile([65, L], F32, tag="ops")
            opsum_b = o_ps.tile([65, L], F32, tag="ops")
            heads = (
                (ja, ma, 0, opsum_a, vps[0], (0, 0)),
                (jb, mb_n, 64, opsum_b, vps[1], (64, 0)),
            )
            for c in range(mmax):
                live = [hd for hd in heads if c < hd[1]]
                stiles = {}
                for j, m, row0, opsum, vp, tpos in live:
                    stiles[j] = s_ps.tile([128, L], F32, tag="s", name=f"s_{j}_{c}")
                for h in range(2):
                    for j, m, row0, opsum, vp, tpos in live:
                        nc.tensor.matmul(
                            stiles[j][:, h * 512 : (h + 1) * 512],
                            kt[row0 : row0 + 64, c * 128 : (c + 1) * 128],
                            qt[row0 : row0 + 64, h * 512 : (h + 1) * 512],
                            start=True,
                            stop=True,
                            tile_position=tpos,
                        )
                for j, m, row0, opsum, vp, tpos in live:
                    s = stiles[j]
                    pt = pt_p.tile([128, L], MM_DT)
                    col = j * NCH + c
                    nc.scalar.activation(
                        pt[:], s[:], Exp, bias=mb[:, col : col + 1], scale=0.125
                    )
                    vl = vp[:, c * (D + 1) : (c + 1) * (D + 1)]
                    for h in range(2):
                        nc.tensor.matmul(
                            opsum[:, h * 512 : (h + 1) * 512],
                            vl,
                            pt[:, h * 512 : (h + 1) * 512],
                            start=(c == 0),
                            stop=(c == m - 1),
                        )

            # Epilogue per head: transpose [O^T ; denom] back, normalize, store.
            for j, m, row0, opsum, vp, tpos in heads:
                ot = ot_p.tile([65, L], F32)
                nc.vector.tensor_copy(ot[:], opsum[:])
                osb = osb_p.tile([128, NCH * D], F32)
                for g in range(NCH):
                    tt = s_ps.tile([128, 65], F32, tag="s")
                    nc.tensor.transpose(
                        tt[:], ot[:, g * 128 : (g + 1) * 128], ident[0:65, 0:65]
                    )
                    rec = rec_p.tile([128, 1], F32)
                    nc.vector.reciprocal(rec[:], tt[:, 64:65])
                    nc.vector.tensor_scalar_mul(
                        osb[:, g * D : (g + 1) * D], tt[:, 0:64], rec[:]
                    )
                nc.scalar.dma_start(
                    o_d[j].rearrange("(g p) d -> p g d", p=CHUNK),
                    osb[:].rearrange("p (g d) -> p g d", d=D),
                )

    nc.compile()
    return nc


def _plan(valid_lens):
    """Sort heads by valid_len desc, deal round-robin across cores.

    Returns (assign [NCORES, SLOTS] head indices, m_list [SLOTS] chunk counts).
    """
    order = np.argsort(-valid_lens, kind="stable")
    assign = order.reshape(SLOTS, NCORES).T  # [core, slot]
    m_list = []
    for j in range(SLOTS):
        vmax = int(valid_lens[assign[:, j]].max())
        m_list.append(min(NCH, max(1, math.ceil(vmax / CHUNK))))
    return assign, m_list


def _run(queries, keys, values, valid_lens, trace=False):
    queries = np.ascontiguousarray(np.asarray(queries, dtype=np.float32))
    keys = np.ascontiguousarray(np.asarray(keys, dtype=np.float32))
    values = np.ascontiguousarray(np.asarray(values, dtype=np.float32))
    valid_lens = np.asarray(valid_lens, dtype=np.int32)

    assign, m_list = _plan(valid_lens)

    key = tuple(m_list)
    nc = _program_cache.get(key)
    if nc is None:
        nc = _build_program(m_list)
        _program_cache[key] = nc

    kk = np.arange(L, dtype=np.int64)
    in_maps = []
    for i in range(NCORES):
        heads = assign[i]
        mask = np.where(
            kk[None, :] < valid_lens[heads][:, None], 0.0, MASK_VALUE
        ).astype(np.float32)  # [SLOTS, L]
        # mb[p, j*NCH+c] = mask for key index c*128+p of slot j.
        mb = np.transpose(mask.reshape(SLOTS, NCH, CHUNK), (2, 0, 1)).reshape(
            CHUNK, SLOTS * NCH
        )
        qh, kh = queries[heads], keys[heads]
        qp = np.concatenate([qh[0::2], qh[1::2]], axis=2)  # [SLOTS//2, L, 128]
        kp = np.concatenate([kh[0::2], kh[1::2]], axis=2)
        in_maps.append(
            {
                "q": np.ascontiguousarray(qp),
                "k": np.ascontiguousarray(kp),
                "v": values[heads],
                "mb": np.ascontiguousarray(mb),
            }
        )

    res = run_bass_kernel_spmd(nc, in_maps, list(range(NCORES)), trace=trace)

    out = np.empty((BH, L, D), dtype=np.float32)
    for i in range(NCORES):
        out[assign[i]] = res.results[i]["o"]

    # valid_len == 0: reference softmaxes an all-masked row -> uniform weights.
    for h in np.nonzero(valid_lens == 0)[0]:
        out[h] = values[h].mean(axis=0, keepdims=True)

    return out, res


def kernel(queries, keys, values, valid_lens):
    out, _ = _run(queries, keys, values, valid_lens)
    return out


# revision 15
# speedup vs baseline: 1.2526x; 1.2526x over previous
"""Bass/Tile kernel for masked dot-product attention on 8 Trainium2 cores.

Problem: queries/keys/values [128, 1024, 64] fp32, valid_lens [128] int32.
  out[b] = softmax(mask(Q K^T / 8, valid_lens[b])) @ V

Strategy:
  * Shard the 128 batch*heads across 8 cores, 16 head-slots per core.
    Heads are sorted by valid_len (descending) and dealt round-robin so
    every core gets the same per-slot chunk count -> one SPMD program.
  * Per head, only ceil(valid_len/128) key chunks contribute (the rest are
    fully masked -> softmax weight exactly 0), so the program is
    specialized to skip them (~45% of the work for uniform valid_lens).
  * Layout: compute S^T = K Q^T chunkwise on the PE ([128 k x 1024 q]),
    so the PV matmul can consume P^T directly as the moving operand.
    Masking + 1/sqrt(d) scaling + exp run as a single ScalarE activation
    (bias = per-partition mask column of 0 / -1e6; no max subtraction is
    needed: scores are bounded and exp(-1e6) underflows to exactly 0,
    matching the fp32 reference).
  * Softmax denominators come free: a ones-column is appended to V, so
    the PV accumulation produces [O^T ; sum_k P^T] in one pass.
    Normalization happens after a final PE transpose, where the
    denominator is a per-partition scalar.
  * Heads with valid_len == 0 (reference: uniform attention) are fixed up
    on the host with the exact reference semantics (mean of V).
"""

import math
from contextlib import ExitStack

import numpy as np

import concourse.bass as bass  # noqa: F401  (engine namespaces live on the nc)
import concourse.mybir as mybir
import concourse.tile as tile
from concourse import bacc
from concourse.bass_utils import run_bass_kernel_spmd
from concourse.masks import make_identity

BH, L, D = 128, 1024, 64
NCORES = 8
SLOTS = BH // NCORES  # 16
CHUNK = 128
NCH = L // CHUNK  # 8
MASK_VALUE = -1000000.0
F32 = mybir.dt.float32
MM_DT = mybir.dt.float16  # 1 cyc/row on PE, ~2^-11 operand quantization

_program_cache: dict = {}


def _build_program(m_list):
    nc = bacc.Bacc("TRN2", target_bir_lowering=False, debug=False)
    NP = SLOTS // 2
    q_d = nc.dram_tensor("q", [NP, L, 2 * D], F32, kind="ExternalInput").ap()
    k_d = nc.dram_tensor("k", [NP, L, 2 * D], F32, kind="ExternalInput").ap()
    v_d = nc.dram_tensor("v", [SLOTS, L, D], F32, kind="ExternalInput").ap()
    qscr = [
        nc.dram_tensor(f"qscr{p}", [L, 2 * D], MM_DT).ap() for p in range(NP)
    ]
    kscr = [
        nc.dram_tensor(f"kscr{p}", [L, 2 * D], MM_DT).ap() for p in range(NP)
    ]
    mb_d = nc.dram_tensor("mb", [CHUNK, SLOTS * NCH], F32, kind="ExternalInput").ap()
    o_d = nc.dram_tensor("o", [SLOTS, L, D], F32, kind="ExternalOutput").ap()

    Exp = mybir.ActivationFunctionType.Exp

    with tile.TileContext(nc) as tc, ExitStack() as ctx:
        const = ctx.enter_context(tc.tile_pool(name="const", bufs=1))
        ident = const.tile([128, 128], F32)
        make_identity(nc, ident)
        mb = const.tile([CHUNK, SLOTS * NCH], F32)
        nc.sync.dma_start(mb[:], mb_d[:])
        ones = const.tile([128, 1], F32)
        nc.gpsimd.memset(ones[:], 1.0)

        qpf_p = ctx.enter_context(tc.tile_pool(name="qpf", bufs=2))
        qpb_p = ctx.enter_context(tc.tile_pool(name="qpb", bufs=2))
        qt_p = ctx.enter_context(tc.tile_pool(name="qt", bufs=2))
        kt_p = ctx.enter_context(tc.tile_pool(name="kt", bufs=2))
        vnat_p = ctx.enter_context(tc.tile_pool(name="vnat", bufs=3))
        vp_p = ctx.enter_context(tc.tile_pool(name="vp", bufs=3))
        pt_p = ctx.enter_context(tc.tile_pool(name="pt", bufs=3))
        ot_p = ctx.enter_context(tc.tile_pool(name="ot", bufs=2))
        osb_p = ctx.enter_context(tc.tile_pool(name="osb", bufs=4))
        rec_p = ctx.enter_context(tc.tile_pool(name="rec", bufs=4))

        # PSUM: 8 banks. "s": S^T tiles + epilogue transposes (2 x 2 banks);
        # "ops": PV accumulators (2 x 2 banks).
        s_ps = ctx.enter_context(tc.tile_pool(name="s", bufs=2, space="PSUM"))
        o_ps = ctx.enter_context(tc.tile_pool(name="ops", bufs=2, space="PSUM"))

        # Dense matmul burst to flip the PE HAM clock-gate to full rate
        # (~3.4us of contiguous activity required) before real work starts.
        warm = const.tile([128, 512], MM_DT, tag="warm")
        nc.gpsimd.memset(warm[:], 0.5)
        wps = o_ps.tile([128, 512], F32, tag="ops")  # noqa
        for i in range(24):
            nc.tensor.matmul(
                wps[:], warm[:, 0:128], warm[:], start=True, stop=True
            )

        def transposed_load(src_pair, scr, nrows, tag):
            """DRAM f32 [nrows, 128] -> SBUF fp16 [128, nrows] transposed.

            Strided load -> DVE fp16 cast -> DRAM fp16 scratch -> XBAR DMA
            transpose. Row 0-63 of the result = head A's d dims, 64-127 = B's.
            """
            nch = nrows // CHUNK
            pf = qpf_p.tile([128, L * 2], F32, tag="pf")
            nc.sync.dma_start(
                pf[:, 0 : nch * 2 * D].rearrange("p (c d) -> p c d", d=2 * D),
                src_pair[0:nrows].rearrange("(c p) d -> p c d", p=CHUNK),
            )
            pb = qpb_p.tile([128, L * 2], MM_DT, tag="pb")
            nc.vector.tensor_copy(pb[:, 0 : nch * 2 * D], pf[:, 0 : nch * 2 * D])
            nc.sync.dma_start(
                scr[0:nrows].rearrange("(c p) d -> p c d", p=CHUNK),
                pb[:, 0 : nch * 2 * D].rearrange("p (c d) -> p c d", d=2 * D),
            )
            t = qt_p.tile([128, L], MM_DT, tag=tag)
            nc.sync.dma_start_transpose(t[:, 0:nrows], scr[0:nrows])
            return t

        for p in range(SLOTS // 2):
            ja, jb = 2 * p, 2 * p + 1
            ma, mb_n = m_list[ja], m_list[jb]
            mmax = max(ma, mb_n)

            qt = transposed_load(q_d[p], qscr[p], L, "qt")
            kt = transposed_load(k_d[p], kscr[p], mmax * CHUNK, "kt")

            # V (fp16 cast) with appended ones-column, per head.
            vps = []
            for j, m in ((ja, ma), (jb, mb_n)):
                vnat = vnat_p.tile([128, NCH * D], F32, tag="vnat")
                nc.sync.dma_start(
                    vnat[:, 0 : m * D].rearrange("p (c d) -> p c d", d=D),
                    v_d[j, 0 : m * CHUNK].rearrange("(c p) d -> p c d", p=CHUNK),
                )
                vp = vp_p.tile([128, NCH * (D + 1)], MM_DT, tag="vp")
                nc.vector.tensor_copy(
                    vp[:].rearrange("p (c e) -> p c e", e=D + 1)[:, 0:m, 0:D],
                    vnat[:, 0 : m * D].rearrange("p (c d) -> p c d", d=D),
                )
                for c in range(m):
                    base = c * (D + 1)
                    nc.vector.tensor_copy(vp[:, base + D : base + D + 1], ones[:])
                vps.append(vp)

            # Main loop: row-tiled paired QK^T, exp, PV accumulation.
            opsum_a = o_ps.tile([65, L], F32, tag="ops")
            opsum_b = o_ps.tile([65, L], F32, tag="ops")
            heads = (
                (ja, ma, 0, opsum_a, vps[0], (0, 0)),
                (jb, mb_n, 64, opsum_b, vps[1], (64, 0)),
            )
            for c in range(mmax):
                live = [hd for hd in heads if c < hd[1]]
                stiles = {}
                for j, m, row0, opsum, vp, tpos in live:
                    stiles[j] = s_ps.tile([128, L], F32, tag="s", name=f"s_{j}_{c}")
                for h in range(2):
                    for j, m, row0, opsum, vp, tpos in live:
                        nc.tensor.matmul(
                            stiles[j][:, h * 512 : (h + 1) * 512],
                            kt[row0 : row0 + 64, c * 128 : (c + 1) * 128],
                            qt[row0 : row0 + 64, h * 512 : (h + 1) * 512],
                            start=True,
                            stop=True,
                            tile_position=tpos,
                        )
                for j, m, row0, opsum, vp, tpos in live:
                    s = stiles[j]
                    pt = pt_p.tile([128, L], MM_DT)
                    col = j * NCH + c
                    nc.scalar.activation(
                        pt[:], s[:], Exp, bias=mb[:, col : col + 1], scale=0.125
                    )
                    vl = vp[:, c * (D + 1) : (c + 1) * (D + 1)]
                    for h in range(2):
                        nc.tensor.matmul(
                            opsum[:, h * 512 : (h + 1) * 512],
                            vl,
                            pt[:, h * 512 : (h + 1) * 512],
                            start=(c == 0),
                            stop=(c == m - 1),
                        )

            # Epilogue per head: transpose [O^T ; denom] back, normalize, store.
            for j, m, row0, opsum, vp, tpos in heads:
                ot = ot_p.tile([65, L], F32)
                nc.vector.tensor_copy(ot[:], opsum[:])
                osb = osb_p.tile([128, NCH * D], F32)
                for g in range(NCH):
                    tt = o_ps.tile([128, 65], F32, tag="ops")
                    nc.tensor.transpose(
                        tt[:], ot[:, g * 128 : (g + 1) * 128], ident[0:65, 0:65]
                    )
                    rec = rec_p.tile([128, 1], F32)
                    nc.vector.reciprocal(rec[:], tt[:, 64:65])
                    nc.vector.tensor_scalar_mul(
                        osb[:, g * D : (g + 1) * D], tt[:, 0:64], rec[:]
                    )
                nc.scalar.dma_start(
                    o_d[j].rearrange("(g p) d -> p g d", p=CHUNK),
                    osb[:].rearrange("p (g d) -> p g d", d=D),
                )

    nc.compile()
    return nc


def _plan(valid_lens):
    """Sort heads by valid_len desc, deal round-robin across cores.

    Returns (assign [NCORES, SLOTS] head indices, m_list [SLOTS] chunk counts).
    """
    order = np.argsort(-valid_lens, kind="stable")
    assign = order.reshape(SLOTS, NCORES).T  # [core, slot]
    m_list = []
    for j in range(SLOTS):
        vmax = int(valid_lens[assign[:, j]].max())
        m_list.append(min(NCH, max(1, math.ceil(vmax / CHUNK))))
    return assign, m_list


def _run(queries, keys, values, valid_lens, trace=False):
    queries = np.ascontiguousarray(np.asarray(queries, dtype=np.float32))
    keys = np.ascontiguousarray(np.asarray(keys, dtype=np.float32))
    values = np.ascontiguousarray(np.asarray(values, dtype=np.float32))
    valid_lens = np.asarray(valid_lens, dtype=np.int32)

    assign, m_list = _plan(valid_lens)

    key = tuple(m_list)
    nc = _program_cache.get(key)
    if nc is None:
        nc = _build_program(m_list)
        _program_cache[key] = nc

    kk = np.arange(L, dtype=np.int64)
    in_maps = []
    for i in range(NCORES):
        heads = assign[i]
        mask = np.where(
            kk[None, :] < valid_lens[heads][:, None], 0.0, MASK_VALUE
        ).astype(np.float32)  # [SLOTS, L]
        # mb[p, j*NCH+c] = mask for key index c*128+p of slot j.
        mb = np.transpose(mask.reshape(SLOTS, NCH, CHUNK), (2, 0, 1)).reshape(
            CHUNK, SLOTS * NCH
        )
        qh, kh = queries[heads], keys[heads]
        qp = np.concatenate([qh[0::2], qh[1::2]], axis=2)  # [SLOTS//2, L, 128]
        kp = np.concatenate([kh[0::2], kh[1::2]], axis=2)
        in_maps.append(
            {
                "q": np.ascontiguousarray(qp),
                "k": np.ascontiguousarray(kp),
                "v": values[heads],
                "mb": np.ascontiguousarray(mb),
            }
        )

    res = run_bass_kernel_spmd(nc, in_maps, list(range(NCORES)), trace=trace)

    out = np.empty((BH, L, D), dtype=np.float32)
    for i in range(NCORES):
        out[assign[i]] = res.results[i]["o"]

    # valid_len == 0: reference softmaxes an all-masked row -> uniform weights.
    for h in np.nonzero(valid_lens == 0)[0]:
        out[h] = values[h].mean(axis=0, keepdims=True)

    return out, res


def kernel(queries, keys, values, valid_lens):
    out, _ = _run(queries, keys, values, valid_lens)
    return out


# revision 16
# speedup vs baseline: 1.3269x; 1.0594x over previous
"""Bass/Tile kernel for masked dot-product attention on 8 Trainium2 cores.

Problem: queries/keys/values [128, 1024, 64] fp32, valid_lens [128] int32.
  out[b] = softmax(mask(Q K^T / 8, valid_lens[b])) @ V

Strategy:
  * Shard the 128 batch*heads across 8 cores, 16 head-slots per core.
    Heads are sorted by valid_len (descending) and dealt round-robin so
    every core gets the same per-slot chunk count -> one SPMD program.
  * Per head, only ceil(valid_len/128) key chunks contribute (the rest are
    fully masked -> softmax weight exactly 0), so the program is
    specialized to skip them (~45% of the work for uniform valid_lens).
  * Layout: compute S^T = K Q^T chunkwise on the PE ([128 k x 1024 q]),
    so the PV matmul can consume P^T directly as the moving operand.
    Masking + 1/sqrt(d) scaling + exp run as a single ScalarE activation
    (bias = per-partition mask column of 0 / -1e6; no max subtraction is
    needed: scores are bounded and exp(-1e6) underflows to exactly 0,
    matching the fp32 reference).
  * Softmax denominators come free: a ones-column is appended to V, so
    the PV accumulation produces [O^T ; sum_k P^T] in one pass.
    Normalization happens after a final PE transpose, where the
    denominator is a per-partition scalar.
  * Heads with valid_len == 0 (reference: uniform attention) are fixed up
    on the host with the exact reference semantics (mean of V).
"""

import math
from contextlib import ExitStack

import numpy as np

import concourse.bass as bass  # noqa: F401  (engine namespaces live on the nc)
import concourse.mybir as mybir
import concourse.tile as tile
from concourse import bacc
from concourse.bass_utils import run_bass_kernel_spmd
from concourse.masks import make_identity

BH, L, D = 128, 1024, 64
NCORES = 8
SLOTS = BH // NCORES  # 16
CHUNK = 128
NCH = L // CHUNK  # 8
MASK_VALUE = -1000000.0
F32 = mybir.dt.float32
MM_DT = mybir.dt.float16  # 1 cyc/row on PE, ~2^-11 operand quantization

_program_cache: dict = {}


def _build_program(m_list):
    nc = bacc.Bacc("TRN2", target_bir_lowering=False, debug=False)
    NP = SLOTS // 2
    q_d = nc.dram_tensor("q", [NP, L, 2 * D], F32, kind="ExternalInput").ap()
    k_d = nc.dram_tensor("k", [NP, L, 2 * D], F32, kind="ExternalInput").ap()
    v_d = nc.dram_tensor("v", [SLOTS, L, D], F32, kind="ExternalInput").ap()
    qscr = [
        nc.dram_tensor(f"qscr{p}", [L, 2 * D], MM_DT).ap() for p in range(NP)
    ]
    kscr = [
        nc.dram_tensor(f"kscr{p}", [L, 2 * D], MM_DT).ap() for p in range(NP)
    ]
    mb_d = nc.dram_tensor("mb", [CHUNK, SLOTS * NCH], F32, kind="ExternalInput").ap()
    o_d = nc.dram_tensor("o", [SLOTS, L, D], F32, kind="ExternalOutput").ap()

    Exp = mybir.ActivationFunctionType.Exp

    with tile.TileContext(nc) as tc, ExitStack() as ctx:
        const = ctx.enter_context(tc.tile_pool(name="const", bufs=1))
        ident = const.tile([128, 128], F32)
        make_identity(nc, ident)
        mb = const.tile([CHUNK, SLOTS * NCH], F32)
        nc.sync.dma_start(mb[:], mb_d[:])
        ones = const.tile([128, 1], F32)
        nc.gpsimd.memset(ones[:], 1.0)

        qpf_p = ctx.enter_context(tc.tile_pool(name="qpf", bufs=2))
        qpb_p = ctx.enter_context(tc.tile_pool(name="qpb", bufs=2))
        qt_p = ctx.enter_context(tc.tile_pool(name="qt", bufs=3))
        kt_p = ctx.enter_context(tc.tile_pool(name="kt", bufs=3))
        vnat_p = ctx.enter_context(tc.tile_pool(name="vnat", bufs=3))
        vp_p = ctx.enter_context(tc.tile_pool(name="vp", bufs=3))
        pt_p = ctx.enter_context(tc.tile_pool(name="pt", bufs=3))
        ot_p = ctx.enter_context(tc.tile_pool(name="ot", bufs=2))
        osb_p = ctx.enter_context(tc.tile_pool(name="osb", bufs=4))
        rec_p = ctx.enter_context(tc.tile_pool(name="rec", bufs=4))

        # PSUM: 8 banks. "s": S^T tiles + epilogue transposes (2 x 2 banks);
        # "ops": PV accumulators (2 x 2 banks).
        s_ps = ctx.enter_context(tc.tile_pool(name="s", bufs=2, space="PSUM"))
        o_ps = ctx.enter_context(tc.tile_pool(name="ops", bufs=2, space="PSUM"))

        # Dense matmul burst to flip the PE HAM clock-gate to full rate
        # (~3.4us of contiguous activity required) before real work starts.
        warm = const.tile([128, 512], MM_DT, tag="warm")
        nc.gpsimd.memset(warm[:], 0.5)
        wps = o_ps.tile([128, 512], F32, tag="ops")  # noqa
        for i in range(24):
            nc.tensor.matmul(
                wps[:], warm[:, 0:128], warm[:], start=True, stop=True
            )

        def transposed_load(src_pair, scr, nrows, tag):
            """DRAM f32 [nrows, 128] -> SBUF fp16 [128, nrows] transposed.

            Strided load -> DVE fp16 cast -> DRAM fp16 scratch -> XBAR DMA
            transpose. Row 0-63 of the result = head A's d dims, 64-127 = B's.
            """
            nch = nrows // CHUNK
            pf = qpf_p.tile([128, L * 2], F32, tag="pf")
            nc.sync.dma_start(
                pf[:, 0 : nch * 2 * D].rearrange("p (c d) -> p c d", d=2 * D),
                src_pair[0:nrows].rearrange("(c p) d -> p c d", p=CHUNK),
            )
            pb = qpb_p.tile([128, L * 2], MM_DT, tag="pb")
            nc.vector.tensor_copy(pb[:, 0 : nch * 2 * D], pf[:, 0 : nch * 2 * D])
            nc.sync.dma_start(
                scr[0:nrows].rearrange("(c p) d -> p c d", p=CHUNK),
                pb[:, 0 : nch * 2 * D].rearrange("p (c d) -> p c d", d=2 * D),
            )
            t = qt_p.tile([128, L], MM_DT, tag=tag)
            nc.sync.dma_start_transpose(t[:, 0:nrows], scr[0:nrows])
            return t

        for p in range(SLOTS // 2):
            ja, jb = 2 * p, 2 * p + 1
            ma, mb_n = m_list[ja], m_list[jb]
            mmax = max(ma, mb_n)

            qt = transposed_load(q_d[p], qscr[p], L, "qt")
            kt = transposed_load(k_d[p], kscr[p], mmax * CHUNK, "kt")

            # V (fp16 cast) with appended ones-column, per head.
            vps = []
            for j, m in ((ja, ma), (jb, mb_n)):
                vnat = vnat_p.tile([128, NCH * D], F32, tag="vnat")
                nc.sync.dma_start(
                    vnat[:, 0 : m * D].rearrange("p (c d) -> p c d", d=D),
                    v_d[j, 0 : m * CHUNK].rearrange("(c p) d -> p c d", p=CHUNK),
                )
                vp = vp_p.tile([128, NCH * (D + 1)], MM_DT, tag="vp")
                nc.vector.tensor_copy(
                    vp[:].rearrange("p (c e) -> p c e", e=D + 1)[:, 0:m, 0:D],
                    vnat[:, 0 : m * D].rearrange("p (c d) -> p c d", d=D),
                )
                for c in range(m):
                    base = c * (D + 1)
                    nc.vector.tensor_copy(vp[:, base + D : base + D + 1], ones[:])
                vps.append(vp)

            # Main loop: row-tiled paired QK^T, exp, PV accumulation.
            opsum_a = o_ps.tile([65, L], F32, tag="ops")
            opsum_b = o_ps.tile([65, L], F32, tag="ops")
            heads = (
                (ja, ma, 0, opsum_a, vps[0], (0, 0)),
                (jb, mb_n, 64, opsum_b, vps[1], (64, 0)),
            )
            for c in range(mmax):
                live = [hd for hd in heads if c < hd[1]]
                stiles = {}
                for j, m, row0, opsum, vp, tpos in live:
                    stiles[j] = s_ps.tile([128, L], F32, tag="s", name=f"s_{j}_{c}")
                for h in range(2):
                    for j, m, row0, opsum, vp, tpos in live:
                        nc.tensor.matmul(
                            stiles[j][:, h * 512 : (h + 1) * 512],
                            kt[row0 : row0 + 64, c * 128 : (c + 1) * 128],
                            qt[row0 : row0 + 64, h * 512 : (h + 1) * 512],
                            start=True,
                            stop=True,
                            tile_position=tpos,
                        )
                for j, m, row0, opsum, vp, tpos in live:
                    s = stiles[j]
                    pt = pt_p.tile([128, L], MM_DT)
                    col = j * NCH + c
                    nc.scalar.activation(
                        pt[:], s[:], Exp, bias=mb[:, col : col + 1], scale=0.125
                    )
                    vl = vp[:, c * (D + 1) : (c + 1) * (D + 1)]
                    for h in range(2):
                        nc.tensor.matmul(
                            opsum[:, h * 512 : (h + 1) * 512],
                            vl,
                            pt[:, h * 512 : (h + 1) * 512],
                            start=(c == 0),
                            stop=(c == m - 1),
                        )

            # Epilogue per head: transpose [O^T ; denom] back (4 blocks per
            # PSUM bank), normalize, one store.
            for j, m, row0, opsum, vp, tpos in heads:
                ot = ot_p.tile([65, L], F32)
                nc.vector.tensor_copy(ot[:], opsum[:])
                osb = osb_p.tile([128, NCH * D], F32)
                for gg in range(2):
                    tt = o_ps.tile([128, 4 * 65], F32, tag="ops", name=f"tt{j}_{gg}")
                    for g4 in range(4):
                        g = 4 * gg + g4
                        nc.tensor.transpose(
                            tt[:, g4 * 65 : g4 * 65 + 65],
                            ot[:, g * 128 : (g + 1) * 128],
                            ident[0:65, 0:65],
                        )
                    rec = rec_p.tile([128, 4], F32, name=f"rec{j}_{gg}")
                    nc.vector.reciprocal(
                        rec[:], tt[:].rearrange("p (g e) -> p g e", e=65)[:, :, 64]
                    )
                    for g4 in range(4):
                        g = 4 * gg + g4
                        nc.vector.tensor_scalar_mul(
                            osb[:, g * D : (g + 1) * D],
                            tt[:, g4 * 65 : g4 * 65 + 64],
                            rec[:, g4 : g4 + 1],
                        )
                nc.scalar.dma_start(
                    o_d[j].rearrange("(g p) d -> p g d", p=CHUNK),
                    osb[:].rearrange("p (g d) -> p g d", d=D),
                )

    nc.compile()
    return nc


def _plan(valid_lens):
    """Sort heads by valid_len desc, deal round-robin across cores.

    Returns (assign [NCORES, SLOTS] head indices, m_list [SLOTS] chunk counts).
    """
    order = np.argsort(-valid_lens, kind="stable")
    assign = order.reshape(SLOTS, NCORES).T  # [core, slot]
    m_list = []
    for j in range(SLOTS):
        vmax = int(valid_lens[assign[:, j]].max())
        m_list.append(min(NCH, max(1, math.ceil(vmax / CHUNK))))
    return assign, m_list


def _run(queries, keys, values, valid_lens, trace=False):
    queries = np.ascontiguousarray(np.asarray(queries, dtype=np.float32))
    keys = np.ascontiguousarray(np.asarray(keys, dtype=np.float32))
    values = np.ascontiguousarray(np.asarray(values, dtype=np.float32))
    valid_lens = np.asarray(valid_lens, dtype=np.int32)

    assign, m_list = _plan(valid_lens)

    key = tuple(m_list)
    nc = _program_cache.get(key)
    if nc is None:
        nc = _build_program(m_list)
        _program_cache[key] = nc

    kk = np.arange(L, dtype=np.int64)
    in_maps = []
    for i in range(NCORES):
        heads = assign[i]
        mask = np.where(
            kk[None, :] < valid_lens[heads][:, None], 0.0, MASK_VALUE
        ).astype(np.float32)  # [SLOTS, L]
        # mb[p, j*NCH+c] = mask for key index c*128+p of slot j.
        mb = np.transpose(mask.reshape(SLOTS, NCH, CHUNK), (2, 0, 1)).reshape(
            CHUNK, SLOTS * NCH
        )
        qh, kh = queries[heads], keys[heads]
        qp = np.concatenate([qh[0::2], qh[1::2]], axis=2)  # [SLOTS//2, L, 128]
        kp = np.concatenate([kh[0::2], kh[1::2]], axis=2)
        in_maps.append(
            {
                "q": np.ascontiguousarray(qp),
                "k": np.ascontiguousarray(kp),
                "v": values[heads],
                "mb": np.ascontiguousarray(mb),
            }
        )

    res = run_bass_kernel_spmd(nc, in_maps, list(range(NCORES)), trace=trace)

    out = np.empty((BH, L, D), dtype=np.float32)
    for i in range(NCORES):
        out[assign[i]] = res.results[i]["o"]

    # valid_len == 0: reference softmaxes an all-masked row -> uniform weights.
    for h in np.nonzero(valid_lens == 0)[0]:
        out[h] = values[h].mean(axis=0, keepdims=True)

    return out, res


def kernel(queries, keys, values, valid_lens):
    out, _ = _run(queries, keys, values, valid_lens)
    return out


# revision 17
# speedup vs baseline: 1.4528x; 1.0949x over previous
"""Bass/Tile kernel for masked dot-product attention on 8 Trainium2 cores.

Problem: queries/keys/values [128, 1024, 64] fp32, valid_lens [128] int32.
  out[b] = softmax(mask(Q K^T / 8, valid_lens[b])) @ V

Strategy:
  * Shard the 128 batch*heads across 8 cores, 16 head-slots per core.
    Heads are sorted by valid_len (descending) and dealt round-robin so
    every core gets the same per-slot chunk count -> one SPMD program.
  * Per head, only ceil(valid_len/128) key chunks contribute (the rest are
    fully masked -> softmax weight exactly 0), so the program is
    specialized to skip them (~45% of the work for uniform valid_lens).
  * Layout: compute S^T = K Q^T chunkwise on the PE ([128 k x 1024 q]),
    so the PV matmul can consume P^T directly as the moving operand.
    Masking + 1/sqrt(d) scaling + exp run as a single ScalarE activation
    (bias = per-partition mask column of 0 / -1e6; no max subtraction is
    needed: scores are bounded and exp(-1e6) underflows to exactly 0,
    matching the fp32 reference).
  * Softmax denominators come free: a ones-column is appended to V, so
    the PV accumulation produces [O^T ; sum_k P^T] in one pass.
    Normalization happens after a final PE transpose, where the
    denominator is a per-partition scalar.
  * Heads with valid_len == 0 (reference: uniform attention) are fixed up
    on the host with the exact reference semantics (mean of V).
"""

import math
from contextlib import ExitStack

import numpy as np

import concourse.bass as bass  # noqa: F401  (engine namespaces live on the nc)
import concourse.mybir as mybir
import concourse.tile as tile
from concourse import bacc
from concourse.bass_utils import run_bass_kernel_spmd
from concourse.masks import make_identity

BH, L, D = 128, 1024, 64
NCORES = 8
SLOTS = BH // NCORES  # 16
CHUNK = 128
NCH = L // CHUNK  # 8
MASK_VALUE = -1000000.0
F32 = mybir.dt.float32
MM_DT = mybir.dt.float16  # 1 cyc/row on PE, ~2^-11 operand quantization

_program_cache: dict = {}


def _build_program(m_list):
    nc = bacc.Bacc("TRN2", target_bir_lowering=False, debug=False)
    NP = SLOTS // 2
    q_d = nc.dram_tensor("q", [NP, L, 2 * D], F32, kind="ExternalInput").ap()
    k_d = nc.dram_tensor("k", [NP, L, 2 * D], F32, kind="ExternalInput").ap()
    v_d = nc.dram_tensor("v", [SLOTS, L, D], F32, kind="ExternalInput").ap()
    qscr = [
        nc.dram_tensor(f"qscr{p}", [L, 2 * D], MM_DT).ap() for p in range(NP)
    ]
    kscr = [
        nc.dram_tensor(f"kscr{p}", [L, 2 * D], MM_DT).ap() for p in range(NP)
    ]
    mb_d = nc.dram_tensor("mb", [CHUNK, SLOTS * NCH], F32, kind="ExternalInput").ap()
    o_d = nc.dram_tensor("o", [SLOTS, L, D], F32, kind="ExternalOutput").ap()

    Exp = mybir.ActivationFunctionType.Exp

    with tile.TileContext(nc) as tc, ExitStack() as ctx:
        const = ctx.enter_context(tc.tile_pool(name="const", bufs=1))
        ident = const.tile([128, 128], F32)
        make_identity(nc, ident)
        mb = const.tile([CHUNK, SLOTS * NCH], F32)
        nc.sync.dma_start(mb[:], mb_d[:])
        ones = const.tile([128, 1], F32)
        nc.gpsimd.memset(ones[:], 1.0)

        qpf_p = ctx.enter_context(tc.tile_pool(name="qpf", bufs=2))
        qpb_p = ctx.enter_context(tc.tile_pool(name="qpb", bufs=2))
        qt_p = ctx.enter_context(tc.tile_pool(name="qt", bufs=3))
        kt_p = ctx.enter_context(tc.tile_pool(name="kt", bufs=3))
        vnat_p = ctx.enter_context(tc.tile_pool(name="vnat", bufs=3))
        vp_p = ctx.enter_context(tc.tile_pool(name="vp", bufs=3))
        pt_p = ctx.enter_context(tc.tile_pool(name="pt", bufs=3))
        ot_p = ctx.enter_context(tc.tile_pool(name="ot", bufs=2))
        osb_p = ctx.enter_context(tc.tile_pool(name="osb", bufs=4))
        rec_p = ctx.enter_context(tc.tile_pool(name="rec", bufs=4))

        # PSUM: 8 banks. "s": S^T tiles + epilogue transposes (2 x 2 banks);
        # "ops": PV accumulators (2 x 2 banks).
        s_ps = ctx.enter_context(tc.tile_pool(name="s", bufs=2, space="PSUM"))
        o_ps = ctx.enter_context(tc.tile_pool(name="ops", bufs=4, space="PSUM"))

        # Dense matmul burst to flip the PE HAM clock-gate to full rate
        # (~3.4us of contiguous activity required) before real work starts.
        warm = const.tile([128, 512], MM_DT, tag="warm")
        nc.gpsimd.memset(warm[:], 0.5)
        wps = o_ps.tile([128, 512], F32, tag="ops")  # noqa
        for i in range(14):
            nc.tensor.matmul(
                wps[:], warm[:, 0:128], warm[:], start=True, stop=True
            )

        def transposed_load(src_pair, scr, nrows, tag):
            """DRAM f32 [nrows, 128] -> SBUF fp16 [128, nrows] transposed.

            Strided load -> DVE fp16 cast -> DRAM fp16 scratch -> XBAR DMA
            transpose. Row 0-63 of the result = head A's d dims, 64-127 = B's.
            """
            nch = nrows // CHUNK
            pf = qpf_p.tile([128, L * 2], F32, tag="pf")
            nc.sync.dma_start(
                pf[:, 0 : nch * 2 * D].rearrange("p (c d) -> p c d", d=2 * D),
                src_pair[0:nrows].rearrange("(c p) d -> p c d", p=CHUNK),
            )
            pb = qpb_p.tile([128, L * 2], MM_DT, tag="pb")
            nc.vector.tensor_copy(pb[:, 0 : nch * 2 * D], pf[:, 0 : nch * 2 * D])
            nc.sync.dma_start(
                scr[0:nrows].rearrange("(c p) d -> p c d", p=CHUNK),
                pb[:, 0 : nch * 2 * D].rearrange("p (c d) -> p c d", d=2 * D),
            )
            t = qt_p.tile([128, L], MM_DT, tag=tag)
            nc.sync.dma_start_transpose(t[:, 0:nrows], scr[0:nrows])
            return t

        def load_pair(p):
            ja, jb = 2 * p, 2 * p + 1
            ma, mb_n = m_list[ja], m_list[jb]
            mmax = max(ma, mb_n)
            qt = transposed_load(q_d[p], qscr[p], L, "qt")
            kt = transposed_load(k_d[p], kscr[p], mmax * CHUNK, "kt")
            vps = []
            for j, m in ((ja, ma), (jb, mb_n)):
                vnat = vnat_p.tile([128, NCH * D], F32, tag="vnat", name=f"vn{j}")
                nc.sync.dma_start(
                    vnat[:, 0 : m * D].rearrange("p (c d) -> p c d", d=D),
                    v_d[j, 0 : m * CHUNK].rearrange("(c p) d -> p c d", p=CHUNK),
                )
                vp = vp_p.tile([128, NCH * (D + 1)], MM_DT, tag="vp", name=f"vp{j}")
                nc.vector.tensor_copy(
                    vp[:].rearrange("p (c e) -> p c e", e=D + 1)[:, 0:m, 0:D],
                    vnat[:, 0 : m * D].rearrange("p (c d) -> p c d", d=D),
                )
                for c in range(m):
                    base = c * (D + 1)
                    nc.vector.tensor_copy(vp[:, base + D : base + D + 1], ones[:])
                vps.append(vp)
            return qt, kt, vps

        loaded = load_pair(0)
        for p in range(SLOTS // 2):
            ja, jb = 2 * p, 2 * p + 1
            ma, mb_n = m_list[ja], m_list[jb]
            mmax = max(ma, mb_n)
            qt, kt, vps = loaded
            if p + 1 < SLOTS // 2:
                loaded = load_pair(p + 1)

            # Main loop: row-tiled paired QK^T, exp, PV accumulation.
            opsum_a = [
                o_ps.tile([65, 512], F32, tag="ops", name=f"opsa{p}_{h}")
                for h in range(2)
            ]
            opsum_b = [
                o_ps.tile([65, 512], F32, tag="ops", name=f"opsb{p}_{h}")
                for h in range(2)
            ]
            heads = (
                (ja, ma, 0, opsum_a, vps[0], (0, 0)),
                (jb, mb_n, 64, opsum_b, vps[1], (64, 0)),
            )
            for c in range(mmax):
                live = [hd for hd in heads if c < hd[1]]
                stiles = {}
                for j, m, row0, opsum, vp, tpos in live:
                    stiles[j] = s_ps.tile([128, L], F32, tag="s", name=f"s_{j}_{c}")
                for h in range(2):
                    for j, m, row0, opsum, vp, tpos in live:
                        nc.tensor.matmul(
                            stiles[j][:, h * 512 : (h + 1) * 512],
                            kt[row0 : row0 + 64, c * 128 : (c + 1) * 128],
                            qt[row0 : row0 + 64, h * 512 : (h + 1) * 512],
                            start=True,
                            stop=True,
                            tile_position=tpos,
                        )
                for j, m, row0, opsum, vp, tpos in live:
                    s = stiles[j]
                    pt = pt_p.tile([128, L], MM_DT)
                    col = j * NCH + c
                    nc.scalar.activation(
                        pt[:], s[:], Exp, bias=mb[:, col : col + 1], scale=0.125
                    )
                    vl = vp[:, c * (D + 1) : (c + 1) * (D + 1)]
                    for h in range(2):
                        nc.tensor.matmul(
                            opsum[h][:],
                            vl,
                            pt[:, h * 512 : (h + 1) * 512],
                            start=(c == 0),
                            stop=(c == m - 1),
                        )

            # Epilogue per head: transpose [O^T ; denom] back (4 blocks per
            # PSUM bank), normalize, one store.
            for j, m, row0, opsum, vp, tpos in heads:
                ot = ot_p.tile([65, L], F32)
                for h in range(2):
                    nc.vector.tensor_copy(
                        ot[:, h * 512 : (h + 1) * 512], opsum[h][:]
                    )
                osb = osb_p.tile([128, NCH * D], F32)
                for gg in range(2):
                    tt = o_ps.tile([128, 4 * 65], F32, tag="ops", name=f"tt{j}_{gg}_{p}")
                    for g4 in range(4):
                        g = 4 * gg + g4
                        nc.tensor.transpose(
                            tt[:, g4 * 65 : g4 * 65 + 65],
                            ot[:, g * 128 : (g + 1) * 128],
                            ident[0:65, 0:65],
                        )
                    rec = rec_p.tile([128, 4], F32, name=f"rec{j}_{gg}")
                    nc.vector.reciprocal(
                        rec[:], tt[:].rearrange("p (g e) -> p g e", e=65)[:, :, 64]
                    )
                    for g4 in range(4):
                        g = 4 * gg + g4
                        nc.vector.tensor_scalar_mul(
                            osb[:, g * D : (g + 1) * D],
                            tt[:, g4 * 65 : g4 * 65 + 64],
                            rec[:, g4 : g4 + 1],
                        )
                nc.scalar.dma_start(
                    o_d[j].rearrange("(g p) d -> p g d", p=CHUNK),
                    osb[:].rearrange("p (g d) -> p g d", d=D),
                )

    nc.compile()
    return nc


def _plan(valid_lens):
    """Sort heads by valid_len desc, deal round-robin across cores.

    Returns (assign [NCORES, SLOTS] head indices, m_list [SLOTS] chunk counts).
    """
    order = np.argsort(-valid_lens, kind="stable")
    assign = order.reshape(SLOTS, NCORES).T  # [core, slot]
    m_list = []
    for j in range(SLOTS):
        vmax = int(valid_lens[assign[:, j]].max())
        m_list.append(min(NCH, max(1, math.ceil(vmax / CHUNK))))
    return assign, m_list


def _run(queries, keys, values, valid_lens, trace=False):
    queries = np.ascontiguousarray(np.asarray(queries, dtype=np.float32))
    keys = np.ascontiguousarray(np.asarray(keys, dtype=np.float32))
    values = np.ascontiguousarray(np.asarray(values, dtype=np.float32))
    valid_lens = np.asarray(valid_lens, dtype=np.int32)

    assign, m_list = _plan(valid_lens)

    key = tuple(m_list)
    nc = _program_cache.get(key)
    if nc is None:
        nc = _build_program(m_list)
        _program_cache[key] = nc

    kk = np.arange(L, dtype=np.int64)
    in_maps = []
    for i in range(NCORES):
        heads = assign[i]
        mask = np.where(
            kk[None, :] < valid_lens[heads][:, None], 0.0, MASK_VALUE
        ).astype(np.float32)  # [SLOTS, L]
        # mb[p, j*NCH+c] = mask for key index c*128+p of slot j.
        mb = np.transpose(mask.reshape(SLOTS, NCH, CHUNK), (2, 0, 1)).reshape(
            CHUNK, SLOTS * NCH
        )
        qh, kh = queries[heads], keys[heads]
        qp = np.concatenate([qh[0::2], qh[1::2]], axis=2)  # [SLOTS//2, L, 128]
        kp = np.concatenate([kh[0::2], kh[1::2]], axis=2)
        in_maps.append(
            {
                "q": np.ascontiguousarray(qp),
                "k": np.ascontiguousarray(kp),
                "v": values[heads],
                "mb": np.ascontiguousarray(mb),
            }
        )

    res = run_bass_kernel_spmd(nc, in_maps, list(range(NCORES)), trace=trace)

    out = np.empty((BH, L, D), dtype=np.float32)
    for i in range(NCORES):
        out[assign[i]] = res.results[i]["o"]

    # valid_len == 0: reference softmaxes an all-masked row -> uniform weights.
    for h in np.nonzero(valid_lens == 0)[0]:
        out[h] = values[h].mean(axis=0, keepdims=True)

    return out, res


def kernel(queries, keys, values, valid_lens):
    out, _ = _run(queries, keys, values, valid_lens)
    return out
